# revision 51
# baseline (speedup 1.0000x reference)
"""Trainium2 Bass kernel for GQA attention with RoPE (dense transformer block).

Strategy (8-way tensor parallel over heads):
  - Each of the 8 NeuronCores gets 4 Q heads + 1 KV head (KV head c, Q heads
    4c..4c+3). Host sums the 8 partial outputs after wo ("all-reduce").
  - QKV projection runs as fp8(e4m3, TRN max-240) hi/lo split matmuls in
    DoubleRow perf mode: x*SX = xh + xl, w*SW = wh + wl (hi and residual lo
    share one scale so all three cross terms accumulate into one PSUM bank);
    out = xh@wh + xl@wh + xh@wl, rescaled 2^-16 on the PSUM->SBUF copy.
    3 DoubleRow matmuls per 2 k-tiles replace 2 fp32r matmuls (25% fewer PE
    cycles); dropped xl@wl term costs ~0.3% relative error.
  - RoPE on the vector engine in bf16 (4x DVE mode); Q stays SBUF-resident.
  - Attention in bf16: scoresT = K^T.T @ Q^T per 128-key block with
    fine-grained causal skipping (partial moving operands on diagonal
    blocks), exp on scalar engine, single [128,128] tri mask multiply on the
    diagonal 128-col slice only, PV and ones-rowsum accumulated in PSUM.
  - wo projection also fp8 hi/lo DoubleRow: O is quantized on the fly during
    the softmax normalize (scalar engine writes O_hi, one DVE
    scalar_tensor_tensor writes the residual O_lo).
  - Partial outputs written bf16; host sums in float64.
"""
import math

import numpy as np
import ml_dtypes

import concourse.bass as bass
import concourse.tile as tile
from concourse import bacc, mybir
from concourse.bass_utils import run_bass_kernel_spmd
from concourse.masks import make_identity

B, S, DIM = 2, 2048, 4096
NH, NKV, HD = 32, 8, 128
BS = B * S
NCORES = 8
QH = NH // NCORES          # 4 Q heads per core
DQ = QH * HD               # 512
TCH = 512                  # token chunk
NCH = BS // TCH            # 8 chunks
P = 128
NKP = DIM // 256           # 16 contraction k-tile pairs
NCK = 8                    # weight/matmul chunks per tcn (2 k-pairs each)
KPC = NKP // NCK           # 2 k-pairs per chunk

SX = 32.0                  # fp8 scale for x
SW = 2048.0                # fp8 scale for weights
PSC = 1.0 / (SX * SW)      # PSUM descale for QKV
SO = 16.0                  # fp8 scale for attention output O
PSC_C = 1.0 / (SO * SW)    # PSUM descale for wo projection

F32 = mybir.dt.float32
F8 = mybir.dt.float8e4
BF16 = mybir.dt.bfloat16
AF = mybir.ActivationFunctionType
ALU = mybir.AluOpType
DR = mybir.MatmulPerfMode.DoubleRow

_prog_cache = {}
LAST_RESULTS = None


def _build(variant):
    """variant: 'causal' | 'none' | 'general'"""
    nc = bacc.Bacc(None, target_bir_lowering=False)
    xh8 = nc.dram_tensor("xh8", [DIM, BS], F8, kind="ExternalInput")
    xl8 = nc.dram_tensor("xl8", [DIM, BS], F8, kind="ExternalInput")
    # QKV weights, hi/lo fp8, pre-packed to SBUF layout:
    # col = ((((ck*6+m)*KPC+kpi)*2+hl)*2+s)*P + c
    w8 = nc.dram_tensor("w8", [P, NCK * 6 * KPC * 2 * 2 * P], F8,
                        kind="ExternalInput")
    # wo pairs, hi/lo fp8: row (j*2+hl)*P + p ; col s*DIM + d
    wo8 = nc.dram_tensor("wo8", [4 * P, 2 * DIM], F8, kind="ExternalInput")
    cosb = nc.dram_tensor("cosb", [64, S], BF16, kind="ExternalInput")
    sinb = nc.dram_tensor("sinb", [64, S], BF16, kind="ExternalInput")
    trim = None
    emaskb = None
    if variant == "causal":
        trim = nc.dram_tensor("trim", [P, P], BF16, kind="ExternalInput")
    elif variant == "general":
        emaskb = nc.dram_tensor("emaskb", [S, S], BF16, kind="ExternalInput")
    part = nc.dram_tensor("part", [BS, DIM], BF16, kind="ExternalOutput")

    WCHUNK = 6 * KPC * 2 * 2 * P   # cols per ck chunk = 6144

    with tile.TileContext(nc) as tc:
        with (
            tc.tile_pool(name="const", bufs=1) as constp,
            tc.tile_pool(name="kv", bufs=1) as kvp,
            tc.tile_pool(name="qp", bufs=1) as qp,
        ):
            identb = constp.tile([P, P], BF16)
            onesb = constp.tile([P, P], BF16)
            onesrow = constp.tile([1, P], BF16)
            csp_cos = constp.tile([64, S], BF16)
            csp_sin = constp.tile([64, S], BF16)
            trim_sb = None
            if variant == "causal":
                trim_sb = constp.tile([P, P], BF16)

            # per-batch tiles: phase B(b) must not pick up false semaphore
            # dependencies on the other batch's phase-A writes
            KTb = [kvp.tile([P, S], BF16, name=f"KT{b}") for b in range(B)]
            Vtkb = [kvp.tile([P, S], BF16, name=f"Vt{b}") for b in range(B)]
            qtb = [[qp.tile([P, S], BF16, name=f"qt{h}_{b}")
                    for h in range(QH)] for b in range(B)]

            # ---------------- Phase A: QKV projection + RoPE ----------------
            with (
                tc.tile_pool(name="wqp", bufs=1) as wqp,
                tc.tile_pool(name="xtp", bufs=1) as xtp,
                tc.tile_pool(name="rp", bufs=1) as rp,
                tc.tile_pool(name="psA", bufs=1, space="PSUM") as psA,
            ):
                w_sb = wqp.tile([P, NCK * WCHUNK], F8)

                x_tiles = {}

                def _load_x(tcn, kp0, kp1):
                    if tcn not in x_tiles:
                        x_tiles[tcn] = (
                            xtp.tile([P, 2 * NKP, TCH], F8, tag="xh", bufs=2,
                                     name=f"xh_{tcn}"),
                            xtp.tile([P, 2 * NKP, TCH], F8, tag="xl", bufs=2,
                                     name=f"xl_{tcn}"),
                        )
                    xh_t, xl_t = x_tiles[tcn]
                    for src, dst in ((xh8, xh_t), (xl8, xl_t)):
                        sre = src[:, :].rearrange(
                            "(kp s p) n -> p (kp s) n", p=P, s=2)
                        nc.sync.dma_start(
                            dst[:, kp0 * 2:kp1 * 2, :],
                            sre[:, kp0 * 2:kp1 * 2,
                                tcn * TCH:(tcn + 1) * TCH])

                # startup: per-chunk (w, xh, xl) interleave so the DMA queue
                # stays just ahead of tcn0's compute; constants afterwards
                make_identity(nc, identb)
                nc.vector.memset(onesb[:], 1.0)
                nc.vector.memset(onesrow[:], 1.0)
                MW = KPC * 2 * 2 * P   # w cols per (ck, m) = 1024
                nc.sync.dma_start(w_sb[:, 0:MW], w8[:, 0:MW])
                _load_x(0, 0, 2)
                nc.sync.dma_start(w_sb[:, MW:6 * MW], w8[:, MW:6 * MW])
                for ck in range(1, NCK):
                    _load_x(0, 2 * ck, 2 * ck + 2)
                    nc.sync.dma_start(
                        w_sb[:, ck * WCHUNK:(ck + 1) * WCHUNK],
                        w8[:, ck * WCHUNK:(ck + 1) * WCHUNK])
                    if ck == 5:
                        nc.sync.dma_start(csp_cos[:], cosb[:, :])
                        nc.sync.dma_start(csp_sin[:], sinb[:, :])
                _load_x(1, 0, 4)
                if variant == "causal":
                    nc.sync.dma_start(trim_sb[:], trim[:, :])

                def _wsl(ck, m, kpi, hl):
                    off = ((((ck * 6 + m) * KPC + kpi) * 2 + hl) * 2) * P
                    return w_sb[:, off:off + 2 * P].rearrange(
                        "p (s c) -> p s c", s=2)

                for tcn in range(NCH):
                    if tcn + 1 < NCH:
                        _load_x(tcn + 1, 4 if tcn == 0 else 0, 8)
                        _load_x(tcn + 1, 8, 16)
                    xh_t, xl_t = x_tiles.pop(tcn)
                    acc = [psA.tile([P, TCH], F32, tag=f"acc{m}", bufs=1,
                                    name=f"acc{m}_{tcn}")
                           for m in range(6)]
                    morder = (5, 4, 0, 1, 2, 3) if tcn == NCH - 1 else \
                        (0, 1, 2, 3, 4, 5)
                    for ck in range(NCK):
                        last_ck = ck == NCK - 1
                        for m in morder:
                            for kpi in range(KPC):
                                kp = ck * KPC + kpi
                                rh = xh_t[:, 2 * kp:2 * kp + 2, :]
                                rl = xl_t[:, 2 * kp:2 * kp + 2, :]
                                wh = _wsl(ck, m, kpi, 0)
                                wl = _wsl(ck, m, kpi, 1)
                                st = ck == 0 and kpi == 0
                                sp = last_ck and kpi == KPC - 1
                                nc.tensor.matmul(acc[m][:], wh, rh,
                                                 start=st, stop=False,
                                                 perf_mode=DR)
                                nc.tensor.matmul(acc[m][:], wh, rl,
                                                 start=False, stop=False,
                                                 perf_mode=DR)
                                nc.tensor.matmul(acc[m][:], wl, rh,
                                                 start=False, stop=sp,
                                                 perf_mode=DR)
                            if not last_ck:
                                continue
                            # m finished all chunks: copy out + rope now,
                            # overlapping with m+1's matmuls
                            cs = csp_cos[:, (tcn % 4) * TCH:
                                         (tcn % 4 + 1) * TCH]
                            sn = csp_sin[:, (tcn % 4) * TCH:
                                         (tcn % 4 + 1) * TCH]
                            if m < 5:
                                slo = rp.tile([64, TCH], BF16, tag="slo",
                                              bufs=2, name=f"slo{m}_{tcn}")
                                shi = rp.tile([64, TCH], BF16, tag="shi",
                                              bufs=2, name=f"shi{m}_{tcn}")
                                nc.scalar.activation(slo[:], acc[m][0:64, :],
                                                     AF.Copy, scale=PSC)
                                nc.scalar.activation(shi[:], acc[m][64:P, :],
                                                     AF.Copy, scale=PSC)
                                tb, cb = tcn // 4, tcn % 4
                                if m < QH:
                                    dst = qtb[tb][m][:, cb * TCH:
                                                     (cb + 1) * TCH]
                                else:
                                    dst = KTb[tb][:, cb * TCH:(cb + 1) * TCH]
                                o_lo, o_hi = dst[0:64, :], dst[64:P, :]
                                tA = rp.tile([64, TCH], BF16, tag="tA",
                                             bufs=2, name=f"tA{m}_{tcn}")
                                tB = rp.tile([64, TCH], BF16, tag="tB",
                                             bufs=2, name=f"tB{m}_{tcn}")
                                nc.vector.tensor_mul(tA[:], slo[:], cs)
                                nc.vector.tensor_mul(tB[:], shi[:], sn)
                                nc.vector.tensor_sub(o_lo, tA[:], tB[:])
                                tC = rp.tile([64, TCH], BF16, tag="tC",
                                             bufs=2, name=f"tC{m}_{tcn}")
                                tD = rp.tile([64, TCH], BF16, tag="tD",
                                             bufs=2, name=f"tD{m}_{tcn}")
                                nc.vector.tensor_mul(tC[:], slo[:], sn)
                                nc.vector.tensor_mul(tD[:], shi[:], cs)
                                nc.vector.tensor_add(o_hi, tC[:], tD[:])
                            else:
                                vch = rp.tile([P, TCH], BF16, tag="vch",
                                              bufs=2, name=f"vch_{tcn}")
                                nc.scalar.activation(vch[:], acc[m][:],
                                                     AF.Copy, scale=PSC)
                                for j in range(TCH // P):
                                    tp_ps = psA.tile([P, P], BF16, tag="tp",
                                                     bufs=2,
                                                     name=f"tp_{tcn}_{j}")
                                    nc.tensor.transpose(
                                        tp_ps[:], vch[:, j * P:(j + 1) * P],
                                        identb[:])
                                    g = (tcn % 4) * 4 + j
                                    nc.scalar.copy(
                                        Vtkb[tcn // 4][:, g * P:(g + 1) * P],
                                        tp_ps[:])

            # ------------- Phases B+C per batch -------------
            with (
                tc.tile_pool(name="ebp", bufs=1) as ebp,
                tc.tile_pool(name="obp", bufs=1) as obp,
                tc.tile_pool(name="wop", bufs=1) as wop,
                tc.tile_pool(name="osb", bufs=1) as osbp,
                tc.tile_pool(name="mkp", bufs=1) as mkp,
                tc.tile_pool(name="psC", bufs=1, space="PSUM") as psC,
            ):
                # wo pairs hi/lo: [j][hl] -> [P, 2, DIM] fp8
                wo_sb = [[None, None] for _ in range(2)]
                for j in range(2):
                    for hl in range(2):
                        t2 = wop.tile([P, 2, DIM], F8, tag=f"wo{j}{hl}",
                                      bufs=1, name=f"wo_sb{j}{hl}")
                        r0 = (j * 2 + hl) * P
                        nc.sync.dma_start(
                            t2[:],
                            wo8[r0:r0 + P, :].rearrange(
                                "p (s d) -> p s d", s=2))
                        wo_sb[j][hl] = t2

                Opairs = {}
                c_queue = []

                def emit_C_unit(b, tt, ng, pool=None, tag="pc", bufs=2,
                                tail=False, last=False):
                    # one group = 4 nn output chunks sharing one out-DMA
                    Opair = Opairs[b]
                    ob = obp.tile([P, 4 * TCH], BF16, tag="obc", bufs=4)
                    for q in range(4):
                        nn = ng * 4 + q
                        pp = (pool or psC).tile([P, TCH], F32, tag=tag,
                                                bufs=bufs)
                        for j in range(2):
                            oh = Opair[j][0][:, :, tt * P:(tt + 1) * P]
                            ol = Opair[j][1][:, :, tt * P:(tt + 1) * P]
                            wh = wo_sb[j][0][:, :, nn * TCH:(nn + 1) * TCH]
                            wl = wo_sb[j][1][:, :, nn * TCH:(nn + 1) * TCH]
                            nc.tensor.matmul(pp[:], oh, wh, start=(j == 0),
                                             stop=False, perf_mode=DR)
                            nc.tensor.matmul(pp[:], ol, wh, start=False,
                                             stop=False, perf_mode=DR)
                            nc.tensor.matmul(pp[:], oh, wl, start=False,
                                             stop=(j == 1), perf_mode=DR)
                        dst = ob[:, q * TCH:(q + 1) * TCH]
                        if tail:
                            hf = TCH // 2
                            nc.vector.tensor_scalar_mul(
                                dst[:, 0:hf], pp[:, 0:hf], PSC_C)
                            nc.scalar.activation(dst[:, hf:TCH],
                                                 pp[:, hf:TCH],
                                                 AF.Copy, scale=PSC_C)
                        elif q % 2 == 0:
                            nc.vector.tensor_scalar_mul(dst, pp[:], PSC_C)
                        else:
                            nc.scalar.activation(dst, pp[:], AF.Copy,
                                                 scale=PSC_C)
                        if last:
                            nc.sync.dma_start(
                                part[b * S + tt * P:b * S + (tt + 1) * P,
                                     nn * TCH:(nn + 1) * TCH], dst)
                    if not last:
                        nc.sync.dma_start(
                            part[b * S + tt * P:b * S + (tt + 1) * P,
                                 ng * 4 * TCH:(ng + 1) * 4 * TCH], ob[:])

                def drain_C(k):
                    for _ in range(min(k, len(c_queue))):
                        emit_C_unit(*c_queue.pop(0))

                psB_cm = tc.tile_pool(name="psB", bufs=1, space="PSUM")
                psB = psB_cm.__enter__()
                for b in range(B):
                    # O pairs fp8 hi/lo for this batch: [j][hl] -> [P, 2, S]
                    Opair = [[osbp.tile([P, 2, S], F8, tag=f"op{j}{hl}_{b}",
                                        bufs=1, name=f"op{j}{hl}_{b}")
                              for hl in range(2)] for j in range(2)]
                    Opairs[b] = Opair

                    # ---- Phase B: attention for batch b, C units of the
                    # previous sc-quarter interleaved into the PE stream ----
                    for sc in range(4):
                        for h in range(QH):
                            ntt = 4 * sc + 4 if variant == "causal" else 16
                            o_ps = psB.tile([P, TCH], F32, tag="o", bufs=1,
                                            name=f"o_{b}_{h}_{sc}")
                            # softmax denominators, accumulated as [128, 1]
                            # columns (out-free=1 matmuls are ~free on PE)
                            sT_ps = psB.tile([P, 4], F32, tag="st", bufs=1,
                                             name=f"st_{b}_{h}_{sc}")

                            def _scores(tt):
                                diag = variant == "causal" and tt >= 4 * sc
                                c0 = (tt - 4 * sc) * P if diag else 0
                                scp = psB.tile([P, TCH], F32, tag="sc",
                                               bufs=2)
                                nc.tensor.matmul(
                                    scp[:, c0:TCH],
                                    KTb[b][:, tt * P:(tt + 1) * P],
                                    qtb[b][h][:, sc * TCH + c0:
                                              (sc + 1) * TCH],
                                    start=True, stop=True)
                                return diag, c0, scp

                            pend = _scores(0)
                            for tt in range(ntt):
                                diag, c0, sc_ps = pend
                                if tt + 1 < ntt:
                                    pend = _scores(tt + 1)
                                et = ebp.tile([P, TCH], BF16, tag="et",
                                              bufs=6)
                                nc.scalar.activation(et[:, c0:TCH],
                                                     sc_ps[:, c0:TCH], AF.Exp)
                                if diag:
                                    nc.vector.tensor_mul(
                                        et[:, c0:c0 + P], et[:, c0:c0 + P],
                                        trim_sb[:])
                                elif variant == "general":
                                    mt = mkp.tile([P, TCH], BF16, tag="mg",
                                                  bufs=3)
                                    nc.sync.dma_start(
                                        mt[:],
                                        emaskb[tt * P:(tt + 1) * P,
                                               sc * TCH:(sc + 1) * TCH])
                                    nc.vector.tensor_mul(et[:, :], et[:, :],
                                                         mt[:])
                                nc.tensor.matmul(
                                    o_ps[:, c0:TCH],
                                    Vtkb[b][:, tt * P:(tt + 1) * P],
                                    et[:, c0:TCH],
                                    start=(tt == 0), stop=(tt == ntt - 1))
                                jj0 = c0 // P
                                for jj in range(jj0, 4):
                                    nc.tensor.matmul(
                                        sT_ps[:, jj:jj + 1],
                                        et[:, jj * P:(jj + 1) * P],
                                        onesb[:, 0:1],
                                        start=(tt == 0 and jj == jj0),
                                        stop=(tt == ntt - 1 and jj == 3))
                            # denominators -> [1, 512] row -> broadcast to
                            # all partitions via a K=1 matmul -> reciprocal.
                            # C units drained between steps so the PE never
                            # waits on the DVE copies in this chain.
                            sT_sb = obp.tile([P, 4], BF16, tag="stsb", bufs=2)
                            nc.vector.tensor_copy(sT_sb[:], sT_ps[:])
                            drain_C(1)
                            dT_ps = psB.tile([1, TCH], BF16, tag="dt", bufs=1)
                            for jj in range(4):
                                nc.tensor.matmul(
                                    dT_ps[0:1, jj * P:(jj + 1) * P],
                                    sT_sb[:, jj:jj + 1], identb[:],
                                    start=(jj == 0), stop=(jj == 3),
                                    is_transpose=True)
                            dT_sb = obp.tile([1, TCH], BF16, tag="dtsb",
                                             bufs=2)
                            nc.vector.tensor_copy(dT_sb[:], dT_ps[:])
                            drain_C(1)
                            db_ps = psB.tile([P, TCH], F32, tag="db", bufs=1)
                            nc.tensor.matmul(db_ps[:], onesrow[:],
                                             dT_sb[0:1, :],
                                             start=True, stop=True)
                            rec = obp.tile([P, TCH], F32, tag="rec", bufs=2)
                            nc.vector.reciprocal(rec[:], db_ps[:])
                            of = obp.tile([P, TCH], F32, tag="of", bufs=2)
                            nc.vector.tensor_mul(of[:], o_ps[:], rec[:])
                            j, sl = h // 2, h % 2
                            dhi = Opair[j][0][:, sl, sc * TCH:(sc + 1) * TCH]
                            dlo = Opair[j][1][:, sl, sc * TCH:(sc + 1) * TCH]
                            nc.vector.tensor_scalar_mul(dhi, of[:], SO)
                            nc.vector.scalar_tensor_tensor(
                                dlo, of[:], SO, dhi,
                                op0=ALU.mult, op1=ALU.subtract)
                            if len(c_queue) > 6:
                                drain_C(1)
                        c_queue.extend(
                            (b, tt, ng) for tt in range(4 * sc, 4 * sc + 4)
                            for ng in range(2))
                psB_cm.__exit__(None, None, None)
                # final C quarter runs alone: deeper PSUM buffering to hide
                # the pp->ob copy recycle latency
                with tc.tile_pool(name="psC2", bufs=1, space="PSUM") as psC2:
                    while c_queue:
                        emit_C_unit(*c_queue.pop(0), pool=psC2, tag="pc2",
                                    bufs=6, tail=True,
                                    last=(len(c_queue) == 0))

    nc.compile()
    return nc


def _get_prog(variant):
    if variant not in _prog_cache:
        _prog_cache[variant] = _build(variant)
    return _prog_cache[variant]


_E4 = ml_dtypes.float8_e4m3


def _q8(a):
    return np.clip(a, -240.0, 240.0).astype(_E4)


def _hilo(a, scale):
    s = (a * scale).astype(np.float32)
    hi = _q8(s)
    lo = _q8(s - hi.astype(np.float32))
    return hi, lo


def prepare(inputs):
    """Host-side sharding prep: returns (variant, program, per-core maps)."""
    x = np.asarray(inputs["x"], dtype=np.float32)
    wq = np.asarray(inputs["wq"], dtype=np.float32)
    wk = np.asarray(inputs["wk"], dtype=np.float32)
    wv = np.asarray(inputs["wv"], dtype=np.float32)
    wo = np.asarray(inputs["wo"], dtype=np.float32)
    fc = np.asarray(inputs["freqs_cos"], dtype=np.float32)
    fs = np.asarray(inputs["freqs_sin"], dtype=np.float32)
    mask = np.asarray(inputs["mask"], dtype=np.float32)

    xT = x.reshape(BS, DIM).T
    xh8, xl8 = _hilo(np.ascontiguousarray(xT), SX)
    perm = np.concatenate([np.arange(0, HD, 2), np.arange(1, HD, 2)])
    wq_p = (wq.reshape(DIM, NH, HD)[:, :, perm] / math.sqrt(HD)).astype(
        np.float32)
    wk_p = wk.reshape(DIM, NKV, HD)[:, :, perm].astype(np.float32)
    cosb = np.ascontiguousarray(fc.T).astype(ml_dtypes.bfloat16)
    sinb = np.ascontiguousarray(fs.T).astype(ml_dtypes.bfloat16)

    if not mask.any():
        variant = "none"
    else:
        il, jl = np.tril_indices(S)
        iu, ju = np.triu_indices(S, 1)
        if np.all(mask[il, jl] == 0.0) and np.all(mask[iu, ju] <= -1e8):
            variant = "causal"
        else:
            variant = "general"

    trim = None
    emaskb = None
    if variant == "causal":
        # trim[t, q] = 1 if key t <= query q (within a diagonal 128-block)
        trim = np.triu(np.ones((P, P), np.float32)).astype(ml_dtypes.bfloat16)
    elif variant == "general":
        with np.errstate(under="ignore", over="ignore"):
            emaskb = np.ascontiguousarray(np.exp(mask).T).astype(
                ml_dtypes.bfloat16)

    nc = _get_prog(variant)

    in_maps = []
    for c in range(NCORES):
        # per-core weights: Q heads 4c..4c+3, KV head c
        w_eff = np.concatenate([
            wq_p[:, c * QH:(c + 1) * QH, :].reshape(DIM, DQ),
            wk_p[:, c, :],
            wv[:, c * HD:(c + 1) * HD],
        ], axis=1)                                    # [DIM, 768]
        wh, wl = _hilo(w_eff, SW)
        # pack to [P, NCK*6*KPC*2*2*P]: col ((((ck*6+m)*KPC+kpi)*2+hl)*2+s)*P+cc
        # value[hl][(2*(ck*KPC+kpi)+s)*128+p, m*128+cc]
        wpk = np.empty((P, NCK * 6 * KPC * 2 * 2 * P), dtype=_E4)
        whl = [wh.reshape(NKP, 2, P, 6, P), wl.reshape(NKP, 2, P, 6, P)]
        # index [kp, s, p, m, cc]
        for ck in range(NCK):
            for m in range(6):
                for kpi in range(KPC):
                    kp = ck * KPC + kpi
                    for hl in range(2):
                        blk = whl[hl][kp, :, :, m, :]        # [s, p, cc]
                        col0 = ((((ck * 6 + m) * KPC + kpi) * 2 + hl) * 2) * P
                        wpk[:, col0:col0 + 2 * P] = (
                            blk.transpose(1, 0, 2).reshape(P, 2 * P))
                    # wpk[p, (s,cc)] = blk[s,p,cc]
        woh, wol = _hilo(wo[c * DQ:(c + 1) * DQ, :], SW)     # [512, 4096]
        wo_pk = np.empty((4 * P, 2 * DIM), dtype=_E4)
        for j in range(2):
            for hl in range(2):
                src = (woh, wol)[hl]
                r0 = (j * 2 + hl) * P
                for s in range(2):
                    wo_pk[r0:r0 + P, s * DIM:(s + 1) * DIM] = \
                        src[(2 * j + s) * P:(2 * j + s + 1) * P, :]
        m = {
            "xh8": xh8,
            "xl8": xl8,
            "w8": wpk,
            "wo8": wo_pk,
            "cosb": cosb,
            "sinb": sinb,
        }
        if variant == "causal":
            m["trim"] = trim
        elif variant == "general":
            m["emaskb"] = emaskb
        in_maps.append(m)
    return variant, nc, in_maps


def kernel(**inputs):
    global LAST_RESULTS
    variant, nc, in_maps = prepare(inputs)
    res = run_bass_kernel_spmd(nc, in_maps, core_ids=list(range(NCORES)))
    LAST_RESULTS = res
    out = np.zeros((BS, DIM), dtype=np.float64)
    for c in range(NCORES):
        out += np.asarray(res.results[c]["part"], dtype=np.float64)
    return out.reshape(B, S, DIM).astype(np.float32)


# revision 59
# speedup vs baseline: 1.0001x; 1.0001x over previous
"""Trainium2 Bass kernel for GQA attention with RoPE (dense transformer block).

Strategy (8-way tensor parallel over heads):
  - Each of the 8 NeuronCores gets 4 Q heads + 1 KV head (KV head c, Q heads
    4c..4c+3). Host sums the 8 partial outputs after wo ("all-reduce").
  - QKV projection runs as fp8(e4m3, TRN max-240) hi/lo split matmuls in
    DoubleRow perf mode: x*SX = xh + xl, w*SW = wh + wl (hi and residual lo
    share one scale so all three cross terms accumulate into one PSUM bank);
    out = xh@wh + xl@wh + xh@wl, rescaled 2^-16 on the PSUM->SBUF copy.
    3 DoubleRow matmuls per 2 k-tiles replace 2 fp32r matmuls (25% fewer PE
    cycles); dropped xl@wl term costs ~0.3% relative error.
  - RoPE on the vector engine in bf16 (4x DVE mode); Q stays SBUF-resident.
  - Attention in bf16: scoresT = K^T.T @ Q^T per 128-key block with
    fine-grained causal skipping (partial moving operands on diagonal
    blocks), exp on scalar engine, single [128,128] tri mask multiply on the
    diagonal 128-col slice only, PV and ones-rowsum accumulated in PSUM.
  - wo projection also fp8 hi/lo DoubleRow: O is quantized on the fly during
    the softmax normalize (scalar engine writes O_hi, one DVE
    scalar_tensor_tensor writes the residual O_lo).
  - Partial outputs written bf16; host sums in float64.
"""
import math

import numpy as np
import ml_dtypes

import concourse.bass as bass
import concourse.tile as tile
from concourse import bacc, mybir
from concourse.bass_utils import run_bass_kernel_spmd
from concourse.masks import make_identity

B, S, DIM = 2, 2048, 4096
NH, NKV, HD = 32, 8, 128
BS = B * S
NCORES = 8
QH = NH // NCORES          # 4 Q heads per core
DQ = QH * HD               # 512
TCH = 512                  # token chunk
NCH = BS // TCH            # 8 chunks
P = 128
NKP = DIM // 256           # 16 contraction k-tile pairs
NCK = 8                    # weight/matmul chunks per tcn (2 k-pairs each)
KPC = NKP // NCK           # 2 k-pairs per chunk

SX = 32.0                  # fp8 scale for x
SW = 2048.0                # fp8 scale for weights
PSC = 1.0 / (SX * SW)      # PSUM descale for QKV
SO = 16.0                  # fp8 scale for attention output O
PSC_C = 1.0 / (SO * SW)    # PSUM descale for wo projection

F32 = mybir.dt.float32
F8 = mybir.dt.float8e4
BF16 = mybir.dt.bfloat16
AF = mybir.ActivationFunctionType
ALU = mybir.AluOpType
DR = mybir.MatmulPerfMode.DoubleRow

_prog_cache = {}
LAST_RESULTS = None


def _build(variant):
    """variant: 'causal' | 'none' | 'general'"""
    nc = bacc.Bacc(None, target_bir_lowering=False)
    xh8 = nc.dram_tensor("xh8", [DIM, BS], F8, kind="ExternalInput")
    xl8 = nc.dram_tensor("xl8", [DIM, BS], F8, kind="ExternalInput")
    # QKV weights, hi/lo fp8, pre-packed to SBUF layout:
    # col = ((((ck*6+m)*KPC+kpi)*2+hl)*2+s)*P + c
    w8 = nc.dram_tensor("w8", [P, NCK * 6 * KPC * 2 * 2 * P], F8,
                        kind="ExternalInput")
    # wo pairs, hi/lo fp8: row (j*2+hl)*P + p ; col s*DIM + d
    wo8 = nc.dram_tensor("wo8", [4 * P, 2 * DIM], F8, kind="ExternalInput")
    cosb = nc.dram_tensor("cosb", [64, S], BF16, kind="ExternalInput")
    sinb = nc.dram_tensor("sinb", [64, S], BF16, kind="ExternalInput")
    trim = None
    emaskb = None
    if variant == "causal":
        trim = nc.dram_tensor("trim", [P, P], BF16, kind="ExternalInput")
    elif variant == "general":
        emaskb = nc.dram_tensor("emaskb", [S, S], BF16, kind="ExternalInput")
    part = nc.dram_tensor("part", [BS, DIM], BF16, kind="ExternalOutput")

    WCHUNK = 6 * KPC * 2 * 2 * P   # cols per ck chunk = 6144

    with tile.TileContext(nc) as tc:
        with (
            tc.tile_pool(name="const", bufs=1) as constp,
            tc.tile_pool(name="kv", bufs=1) as kvp,
            tc.tile_pool(name="qp", bufs=1) as qp,
        ):
            identb = constp.tile([P, P], BF16)
            onesb = constp.tile([P, P], BF16)
            onesrow = constp.tile([1, P], BF16)
            csp_cos = constp.tile([64, S], BF16)
            csp_sin = constp.tile([64, S], BF16)
            trim_sb = None
            if variant == "causal":
                trim_sb = constp.tile([P, P], BF16)

            # per-batch tiles: phase B(b) must not pick up false semaphore
            # dependencies on the other batch's phase-A writes
            KTb = [kvp.tile([P, S], BF16, name=f"KT{b}") for b in range(B)]
            Vtkb = [kvp.tile([P, S], BF16, name=f"Vt{b}") for b in range(B)]
            qtb = [[qp.tile([P, S], BF16, name=f"qt{h}_{b}")
                    for h in range(QH)] for b in range(B)]

            # ---------------- Phase A: QKV projection + RoPE ----------------
            with (
                tc.tile_pool(name="wqp", bufs=1) as wqp,
                tc.tile_pool(name="xtp", bufs=1) as xtp,
                tc.tile_pool(name="rp", bufs=1) as rp,
                tc.tile_pool(name="psA", bufs=1, space="PSUM") as psA,
            ):
                w_sb = wqp.tile([P, NCK * WCHUNK], F8)

                x_tiles = {}

                def _load_x(tcn, kp0, kp1):
                    if tcn not in x_tiles:
                        x_tiles[tcn] = (
                            xtp.tile([P, 2 * NKP, TCH], F8, tag="xh", bufs=2,
                                     name=f"xh_{tcn}"),
                            xtp.tile([P, 2 * NKP, TCH], F8, tag="xl", bufs=2,
                                     name=f"xl_{tcn}"),
                        )
                    xh_t, xl_t = x_tiles[tcn]
                    for src, dst in ((xh8, xh_t), (xl8, xl_t)):
                        sre = src[:, :].rearrange(
                            "(kp s p) n -> p (kp s) n", p=P, s=2)
                        nc.sync.dma_start(
                            dst[:, kp0 * 2:kp1 * 2, :],
                            sre[:, kp0 * 2:kp1 * 2,
                                tcn * TCH:(tcn + 1) * TCH])

                # startup: per-chunk (w, xh, xl) interleave so the DMA queue
                # stays just ahead of tcn0's compute; constants afterwards
                make_identity(nc, identb)
                nc.vector.memset(onesb[:], 1.0)
                nc.vector.memset(onesrow[:], 1.0)
                MW = KPC * 2 * 2 * P   # w cols per (ck, m) = 1024
                nc.sync.dma_start(w_sb[:, 0:MW], w8[:, 0:MW])
                _load_x(0, 0, 2)
                nc.sync.dma_start(w_sb[:, MW:6 * MW], w8[:, MW:6 * MW])
                for ck in range(1, NCK):
                    _load_x(0, 2 * ck, 2 * ck + 2)
                    nc.sync.dma_start(
                        w_sb[:, ck * WCHUNK:(ck + 1) * WCHUNK],
                        w8[:, ck * WCHUNK:(ck + 1) * WCHUNK])
                    if ck == 5:
                        nc.sync.dma_start(csp_cos[:], cosb[:, :])
                        nc.sync.dma_start(csp_sin[:], sinb[:, :])
                _load_x(1, 0, 4)
                if variant == "causal":
                    nc.sync.dma_start(trim_sb[:], trim[:, :])

                def _wsl(ck, m, kpi, hl):
                    off = ((((ck * 6 + m) * KPC + kpi) * 2 + hl) * 2) * P
                    return w_sb[:, off:off + 2 * P].rearrange(
                        "p (s c) -> p s c", s=2)

                for tcn in range(NCH):
                    if tcn + 1 < NCH:
                        _load_x(tcn + 1, 4 if tcn == 0 else 0, 8)
                        _load_x(tcn + 1, 8, 16)
                    xh_t, xl_t = x_tiles.pop(tcn)
                    acc = [psA.tile([P, TCH], F32, tag=f"acc{m}", bufs=1,
                                    name=f"acc{m}_{tcn}")
                           for m in range(6)]
                    morder = (5, 4, 0, 1, 2, 3) if tcn == NCH - 1 else \
                        (0, 1, 2, 3, 4, 5)
                    for ck in range(NCK):
                        last_ck = ck == NCK - 1
                        for m in morder:
                            for kpi in range(KPC):
                                kp = ck * KPC + kpi
                                rh = xh_t[:, 2 * kp:2 * kp + 2, :]
                                rl = xl_t[:, 2 * kp:2 * kp + 2, :]
                                wh = _wsl(ck, m, kpi, 0)
                                wl = _wsl(ck, m, kpi, 1)
                                st = ck == 0 and kpi == 0
                                sp = last_ck and kpi == KPC - 1
                                nc.tensor.matmul(acc[m][:], wh, rh,
                                                 start=st, stop=False,
                                                 perf_mode=DR)
                                nc.tensor.matmul(acc[m][:], wh, rl,
                                                 start=False, stop=False,
                                                 perf_mode=DR)
                                nc.tensor.matmul(acc[m][:], wl, rh,
                                                 start=False, stop=sp,
                                                 perf_mode=DR)
                            if not last_ck:
                                continue
                            # m finished all chunks: copy out + rope now,
                            # overlapping with m+1's matmuls
                            cs = csp_cos[:, (tcn % 4) * TCH:
                                         (tcn % 4 + 1) * TCH]
                            sn = csp_sin[:, (tcn % 4) * TCH:
                                         (tcn % 4 + 1) * TCH]
                            if m < 5:
                                slo = rp.tile([64, TCH], BF16, tag="slo",
                                              bufs=2, name=f"slo{m}_{tcn}")
                                shi = rp.tile([64, TCH], BF16, tag="shi",
                                              bufs=2, name=f"shi{m}_{tcn}")
                                nc.scalar.activation(slo[:], acc[m][0:64, :],
                                                     AF.Copy, scale=PSC)
                                nc.scalar.activation(shi[:], acc[m][64:P, :],
                                                     AF.Copy, scale=PSC)
                                tb, cb = tcn // 4, tcn % 4
                                if m < QH:
                                    dst = qtb[tb][m][:, cb * TCH:
                                                     (cb + 1) * TCH]
                                else:
                                    dst = KTb[tb][:, cb * TCH:(cb + 1) * TCH]
                                o_lo, o_hi = dst[0:64, :], dst[64:P, :]
                                tA = rp.tile([64, TCH], BF16, tag="tA",
                                             bufs=2, name=f"tA{m}_{tcn}")
                                tB = rp.tile([64, TCH], BF16, tag="tB",
                                             bufs=2, name=f"tB{m}_{tcn}")
                                nc.vector.tensor_mul(tA[:], slo[:], cs)
                                nc.vector.tensor_mul(tB[:], shi[:], sn)
                                nc.vector.tensor_sub(o_lo, tA[:], tB[:])
                                tC = rp.tile([64, TCH], BF16, tag="tC",
                                             bufs=2, name=f"tC{m}_{tcn}")
                                tD = rp.tile([64, TCH], BF16, tag="tD",
                                             bufs=2, name=f"tD{m}_{tcn}")
                                nc.vector.tensor_mul(tC[:], slo[:], sn)
                                nc.vector.tensor_mul(tD[:], shi[:], cs)
                                nc.vector.tensor_add(o_hi, tC[:], tD[:])
                            else:
                                vch = rp.tile([P, TCH], BF16, tag="vch",
                                              bufs=2, name=f"vch_{tcn}")
                                nc.scalar.activation(vch[:], acc[m][:],
                                                     AF.Copy, scale=PSC)
                                for j in range(TCH // P):
                                    tp_ps = psA.tile([P, P], BF16, tag="tp",
                                                     bufs=2,
                                                     name=f"tp_{tcn}_{j}")
                                    nc.tensor.transpose(
                                        tp_ps[:], vch[:, j * P:(j + 1) * P],
                                        identb[:])
                                    g = (tcn % 4) * 4 + j
                                    nc.scalar.copy(
                                        Vtkb[tcn // 4][:, g * P:(g + 1) * P],
                                        tp_ps[:])

            # ------------- Phases B+C per batch -------------
            with (
                tc.tile_pool(name="ebp", bufs=1) as ebp,
                tc.tile_pool(name="obp", bufs=1) as obp,
                tc.tile_pool(name="wop", bufs=1) as wop,
                tc.tile_pool(name="osb", bufs=1) as osbp,
                tc.tile_pool(name="mkp", bufs=1) as mkp,
            ):
                # wo pairs hi/lo: [j][hl] -> [P, 2, DIM] fp8
                wo_sb = [[None, None] for _ in range(2)]
                for j in range(2):
                    for hl in range(2):
                        t2 = wop.tile([P, 2, DIM], F8, tag=f"wo{j}{hl}",
                                      bufs=1, name=f"wo_sb{j}{hl}")
                        r0 = (j * 2 + hl) * P
                        nc.sync.dma_start(
                            t2[:],
                            wo8[r0:r0 + P, :].rearrange(
                                "p (s d) -> p s d", s=2))
                        wo_sb[j][hl] = t2

                Opairs = {}
                c_queue = []

                def emit_C_unit(b, tt, ng, pool=None, tag="pc", bufs=2,
                                tail=False, last=False):
                    # one group = 4 nn output chunks sharing one out-DMA
                    Opair = Opairs[b]
                    ob = obp.tile([P, 4 * TCH], BF16, tag="obc", bufs=6)
                    for q in range(4):
                        nn = ng * 4 + q
                        pp = (pool or psC).tile([P, TCH], F32, tag=tag,
                                                bufs=bufs)
                        for j in range(2):
                            oh = Opair[j][0][:, :, tt * P:(tt + 1) * P]
                            ol = Opair[j][1][:, :, tt * P:(tt + 1) * P]
                            wh = wo_sb[j][0][:, :, nn * TCH:(nn + 1) * TCH]
                            wl = wo_sb[j][1][:, :, nn * TCH:(nn + 1) * TCH]
                            nc.tensor.matmul(pp[:], oh, wh, start=(j == 0),
                                             stop=False, perf_mode=DR)
                            nc.tensor.matmul(pp[:], ol, wh, start=False,
                                             stop=False, perf_mode=DR)
                            nc.tensor.matmul(pp[:], oh, wl, start=False,
                                             stop=(j == 1), perf_mode=DR)
                        dst = ob[:, q * TCH:(q + 1) * TCH]
                        if tail:
                            hf = TCH // 2
                            nc.vector.tensor_scalar_mul(
                                dst[:, 0:hf], pp[:, 0:hf], PSC_C)
                            nc.scalar.activation(dst[:, hf:TCH],
                                                 pp[:, hf:TCH],
                                                 AF.Copy, scale=PSC_C)
                        elif q % 2 == 0:
                            nc.vector.tensor_scalar_mul(dst, pp[:], PSC_C)
                        else:
                            nc.scalar.activation(dst, pp[:], AF.Copy,
                                                 scale=PSC_C)
                        if last:
                            nc.sync.dma_start(
                                part[b * S + tt * P:b * S + (tt + 1) * P,
                                     nn * TCH:(nn + 1) * TCH], dst)
                    if not last:
                        nc.sync.dma_start(
                            part[b * S + tt * P:b * S + (tt + 1) * P,
                                 ng * 4 * TCH:(ng + 1) * 4 * TCH], ob[:])

                def drain_C(k):
                    for _ in range(min(k, len(c_queue))):
                        emit_C_unit(*c_queue.pop(0))

                psB_cm = tc.tile_pool(name="psB", bufs=1, space="PSUM")
                psB = psB_cm.__enter__()
                psC_cm = tc.tile_pool(name="psC", bufs=1, space="PSUM")
                psC = psC_cm.__enter__()
                for b in range(B):
                    # O pairs fp8 hi/lo for this batch: [j][hl] -> [P, 2, S]
                    Opair = [[osbp.tile([P, 2, S], F8, tag=f"op{j}{hl}_{b}",
                                        bufs=1, name=f"op{j}{hl}_{b}")
                              for hl in range(2)] for j in range(2)]
                    Opairs[b] = Opair

                    # ---- Phase B: attention for batch b, C units of the
                    # previous sc-quarter interleaved into the PE stream ----
                    for sc in range(4):
                        for h in range(QH):
                            ntt = 4 * sc + 4 if variant == "causal" else 16
                            o_ps = psB.tile([P, TCH], F32, tag="o", bufs=1,
                                            name=f"o_{b}_{h}_{sc}")
                            # softmax denominators, accumulated as [128, 1]
                            # columns (out-free=1 matmuls are ~free on PE)
                            sT_ps = psB.tile([P, 4], F32, tag="st", bufs=1,
                                             name=f"st_{b}_{h}_{sc}")

                            def _scores(tt):
                                diag = variant == "causal" and tt >= 4 * sc
                                c0 = (tt - 4 * sc) * P if diag else 0
                                scp = psB.tile([P, TCH], F32, tag="sc",
                                               bufs=2)
                                nc.tensor.matmul(
                                    scp[:, c0:TCH],
                                    KTb[b][:, tt * P:(tt + 1) * P],
                                    qtb[b][h][:, sc * TCH + c0:
                                              (sc + 1) * TCH],
                                    start=True, stop=True)
                                return diag, c0, scp

                            pend = _scores(0)
                            for tt in range(ntt):
                                diag, c0, sc_ps = pend
                                if tt + 1 < ntt:
                                    pend = _scores(tt + 1)
                                et = ebp.tile([P, TCH], BF16, tag="et",
                                              bufs=6)
                                nc.scalar.activation(et[:, c0:TCH],
                                                     sc_ps[:, c0:TCH], AF.Exp)
                                if diag:
                                    nc.vector.tensor_mul(
                                        et[:, c0:c0 + P], et[:, c0:c0 + P],
                                        trim_sb[:])
                                elif variant == "general":
                                    mt = mkp.tile([P, TCH], BF16, tag="mg",
                                                  bufs=3)
                                    nc.sync.dma_start(
                                        mt[:],
                                        emaskb[tt * P:(tt + 1) * P,
                                               sc * TCH:(sc + 1) * TCH])
                                    nc.vector.tensor_mul(et[:, :], et[:, :],
                                                         mt[:])
                                nc.tensor.matmul(
                                    o_ps[:, c0:TCH],
                                    Vtkb[b][:, tt * P:(tt + 1) * P],
                                    et[:, c0:TCH],
                                    start=(tt == 0), stop=(tt == ntt - 1))
                                jj0 = c0 // P
                                for jj in range(jj0, 4):
                                    nc.tensor.matmul(
                                        sT_ps[:, jj:jj + 1],
                                        et[:, jj * P:(jj + 1) * P],
                                        onesb[:, 0:1],
                                        start=(tt == 0 and jj == jj0),
                                        stop=(tt == ntt - 1 and jj == 3))
                            # denominators -> [1, 512] row -> broadcast to
                            # all partitions via a K=1 matmul -> reciprocal.
                            # C units drained between steps so the PE never
                            # waits on the DVE copies in this chain.
                            sT_sb = obp.tile([P, 4], BF16, tag="stsb", bufs=2)
                            nc.vector.tensor_copy(sT_sb[:], sT_ps[:])
                            drain_C(1)
                            dT_ps = psB.tile([1, TCH], BF16, tag="dt", bufs=1)
                            for jj in range(4):
                                nc.tensor.matmul(
                                    dT_ps[0:1, jj * P:(jj + 1) * P],
                                    sT_sb[:, jj:jj + 1], identb[:],
                                    start=(jj == 0), stop=(jj == 3),
                                    is_transpose=True)
                            dT_sb = obp.tile([1, TCH], BF16, tag="dtsb",
                                             bufs=2)
                            nc.vector.tensor_copy(dT_sb[:], dT_ps[:])
                            drain_C(1)
                            db_ps = psB.tile([P, TCH], F32, tag="db", bufs=1)
                            nc.tensor.matmul(db_ps[:], onesrow[:],
                                             dT_sb[0:1, :],
                                             start=True, stop=True)
                            rec = obp.tile([P, TCH], F32, tag="rec", bufs=2)
                            nc.vector.reciprocal(rec[:], db_ps[:])
                            of = obp.tile([P, TCH], F32, tag="of", bufs=2)
                            nc.vector.tensor_mul(of[:], o_ps[:], rec[:])
                            j, sl = h // 2, h % 2
                            dhi = Opair[j][0][:, sl, sc * TCH:(sc + 1) * TCH]
                            dlo = Opair[j][1][:, sl, sc * TCH:(sc + 1) * TCH]
                            nc.vector.tensor_scalar_mul(dhi, of[:], SO)
                            nc.vector.scalar_tensor_tensor(
                                dlo, of[:], SO, dhi,
                                op0=ALU.mult, op1=ALU.subtract)
                            if len(c_queue) > 6:
                                drain_C(1)
                        c_queue.extend(
                            (b, tt, ng) for tt in range(4 * sc, 4 * sc + 4)
                            for ng in range(2))
                psC_cm.__exit__(None, None, None)
                psB_cm.__exit__(None, None, None)
                # final C quarter runs alone: deeper PSUM buffering to hide
                # the pp->ob copy recycle latency
                with tc.tile_pool(name="psC2", bufs=1, space="PSUM") as psC2:
                    while c_queue:
                        emit_C_unit(*c_queue.pop(0), pool=psC2, tag="pc2",
                                    bufs=8, tail=True,
                                    last=(len(c_queue) == 0))

    nc.compile()
    return nc


def _get_prog(variant):
    if variant not in _prog_cache:
        _prog_cache[variant] = _build(variant)
    return _prog_cache[variant]


_E4 = ml_dtypes.float8_e4m3


def _q8(a):
    return np.clip(a, -240.0, 240.0).astype(_E4)


def _hilo(a, scale):
    s = (a * scale).astype(np.float32)
    hi = _q8(s)
    lo = _q8(s - hi.astype(np.float32))
    return hi, lo


def prepare(inputs):
    """Host-side sharding prep: returns (variant, program, per-core maps)."""
    x = np.asarray(inputs["x"], dtype=np.float32)
    wq = np.asarray(inputs["wq"], dtype=np.float32)
    wk = np.asarray(inputs["wk"], dtype=np.float32)
    wv = np.asarray(inputs["wv"], dtype=np.float32)
    wo = np.asarray(inputs["wo"], dtype=np.float32)
    fc = np.asarray(inputs["freqs_cos"], dtype=np.float32)
    fs = np.asarray(inputs["freqs_sin"], dtype=np.float32)
    mask = np.asarray(inputs["mask"], dtype=np.float32)

    xT = x.reshape(BS, DIM).T
    xh8, xl8 = _hilo(np.ascontiguousarray(xT), SX)
    perm = np.concatenate([np.arange(0, HD, 2), np.arange(1, HD, 2)])
    wq_p = (wq.reshape(DIM, NH, HD)[:, :, perm] / math.sqrt(HD)).astype(
        np.float32)
    wk_p = wk.reshape(DIM, NKV, HD)[:, :, perm].astype(np.float32)
    cosb = np.ascontiguousarray(fc.T).astype(ml_dtypes.bfloat16)
    sinb = np.ascontiguousarray(fs.T).astype(ml_dtypes.bfloat16)

    if not mask.any():
        variant = "none"
    else:
        il, jl = np.tril_indices(S)
        iu, ju = np.triu_indices(S, 1)
        if np.all(mask[il, jl] == 0.0) and np.all(mask[iu, ju] <= -1e8):
            variant = "causal"
        else:
            variant = "general"

    trim = None
    emaskb = None
    if variant == "causal":
        # trim[t, q] = 1 if key t <= query q (within a diagonal 128-block)
        trim = np.triu(np.ones((P, P), np.float32)).astype(ml_dtypes.bfloat16)
    elif variant == "general":
        with np.errstate(under="ignore", over="ignore"):
            emaskb = np.ascontiguousarray(np.exp(mask).T).astype(
                ml_dtypes.bfloat16)

    nc = _get_prog(variant)

    in_maps = []
    for c in range(NCORES):
        # per-core weights: Q heads 4c..4c+3, KV head c
        w_eff = np.concatenate([
            wq_p[:, c * QH:(c + 1) * QH, :].reshape(DIM, DQ),
            wk_p[:, c, :],
            wv[:, c * HD:(c + 1) * HD],
        ], axis=1)                                    # [DIM, 768]
        wh, wl = _hilo(w_eff, SW)
        # pack to [P, NCK*6*KPC*2*2*P]: col ((((ck*6+m)*KPC+kpi)*2+hl)*2+s)*P+cc
        # value[hl][(2*(ck*KPC+kpi)+s)*128+p, m*128+cc]
        wpk = np.empty((P, NCK * 6 * KPC * 2 * 2 * P), dtype=_E4)
        whl = [wh.reshape(NKP, 2, P, 6, P), wl.reshape(NKP, 2, P, 6, P)]
        # index [kp, s, p, m, cc]
        for ck in range(NCK):
            for m in range(6):
                for kpi in range(KPC):
                    kp = ck * KPC + kpi
                    for hl in range(2):
                        blk = whl[hl][kp, :, :, m, :]        # [s, p, cc]
                        col0 = ((((ck * 6 + m) * KPC + kpi) * 2 + hl) * 2) * P
                        wpk[:, col0:col0 + 2 * P] = (
                            blk.transpose(1, 0, 2).reshape(P, 2 * P))
                    # wpk[p, (s,cc)] = blk[s,p,cc]
        woh, wol = _hilo(wo[c * DQ:(c + 1) * DQ, :], SW)     # [512, 4096]
        wo_pk = np.empty((4 * P, 2 * DIM), dtype=_E4)
        for j in range(2):
            for hl in range(2):
                src = (woh, wol)[hl]
                r0 = (j * 2 + hl) * P
                for s in range(2):
                    wo_pk[r0:r0 + P, s * DIM:(s + 1) * DIM] = \
                        src[(2 * j + s) * P:(2 * j + s + 1) * P, :]
        m = {
            "xh8": xh8,
            "xl8": xl8,
            "w8": wpk,
            "wo8": wo_pk,
            "cosb": cosb,
            "sinb": sinb,
        }
        if variant == "causal":
            m["trim"] = trim
        elif variant == "general":
            m["emaskb"] = emaskb
        in_maps.append(m)
    return variant, nc, in_maps


def kernel(**inputs):
    global LAST_RESULTS
    variant, nc, in_maps = prepare(inputs)
    res = run_bass_kernel_spmd(nc, in_maps, core_ids=list(range(NCORES)))
    LAST_RESULTS = res
    out = np.zeros((BS, DIM), dtype=np.float64)
    for c in range(NCORES):
        out += np.asarray(res.results[c]["part"], dtype=np.float64)
    return out.reshape(B, S, DIM).astype(np.float32)


# revision 62
# speedup vs baseline: 1.0025x; 1.0024x over previous
"""Trainium2 Bass kernel for GQA attention with RoPE (dense transformer block).

Strategy (8-way tensor parallel over heads):
  - Each of the 8 NeuronCores gets 4 Q heads + 1 KV head (KV head c, Q heads
    4c..4c+3). Host sums the 8 partial outputs after wo ("all-reduce").
  - QKV projection runs as fp8(e4m3, TRN max-240) hi/lo split matmuls in
    DoubleRow perf mode: x*SX = xh + xl, w*SW = wh + wl (hi and residual lo
    share one scale so all three cross terms accumulate into one PSUM bank);
    out = xh@wh + xl@wh + xh@wl, rescaled 2^-16 on the PSUM->SBUF copy.
    3 DoubleRow matmuls per 2 k-tiles replace 2 fp32r matmuls (25% fewer PE
    cycles); dropped xl@wl term costs ~0.3% relative error.
  - RoPE on the vector engine in bf16 (4x DVE mode); Q stays SBUF-resident.
  - Attention in bf16: scoresT = K^T.T @ Q^T per 128-key block with
    fine-grained causal skipping (partial moving operands on diagonal
    blocks), exp on scalar engine, single [128,128] tri mask multiply on the
    diagonal 128-col slice only, PV and ones-rowsum accumulated in PSUM.
  - wo projection also fp8 hi/lo DoubleRow: O is quantized on the fly during
    the softmax normalize (scalar engine writes O_hi, one DVE
    scalar_tensor_tensor writes the residual O_lo).
  - Partial outputs written bf16; host sums in float64.
"""
import math

import numpy as np
import ml_dtypes

import concourse.bass as bass
import concourse.tile as tile
from concourse import bacc, mybir
from concourse.bass_utils import run_bass_kernel_spmd
from concourse.masks import make_identity

B, S, DIM = 2, 2048, 4096
NH, NKV, HD = 32, 8, 128
BS = B * S
NCORES = 8
QH = NH // NCORES          # 4 Q heads per core
DQ = QH * HD               # 512
TCH = 512                  # token chunk
NCH = BS // TCH            # 8 chunks
P = 128
NKP = DIM // 256           # 16 contraction k-tile pairs
NCK = 8                    # weight/matmul chunks per tcn (2 k-pairs each)
KPC = NKP // NCK           # 2 k-pairs per chunk

SX = 32.0                  # fp8 scale for x
SW = 2048.0                # fp8 scale for weights
PSC = 1.0 / (SX * SW)      # PSUM descale for QKV
SO = 16.0                  # fp8 scale for attention output O
PSC_C = 1.0 / (SO * SW)    # PSUM descale for wo projection

F32 = mybir.dt.float32
F8 = mybir.dt.float8e4
BF16 = mybir.dt.bfloat16
AF = mybir.ActivationFunctionType
ALU = mybir.AluOpType
DR = mybir.MatmulPerfMode.DoubleRow

_prog_cache = {}
LAST_RESULTS = None


def _build(variant):
    """variant: 'causal' | 'none' | 'general'"""
    nc = bacc.Bacc(None, target_bir_lowering=False)
    xh8 = nc.dram_tensor("xh8", [DIM, BS], F8, kind="ExternalInput")
    xl8 = nc.dram_tensor("xl8", [DIM, BS], F8, kind="ExternalInput")
    # QKV weights, hi/lo fp8, pre-packed to SBUF layout:
    # col = ((((ck*6+m)*KPC+kpi)*2+hl)*2+s)*P + c
    w8 = nc.dram_tensor("w8", [P, NCK * 6 * KPC * 2 * 2 * P], F8,
                        kind="ExternalInput")
    # wo pairs, hi/lo fp8: row (j*2+hl)*P + p ; col s*DIM + d
    wo8 = nc.dram_tensor("wo8", [4 * P, 2 * DIM], F8, kind="ExternalInput")
    cosb = nc.dram_tensor("cosb", [64, S], BF16, kind="ExternalInput")
    sinb = nc.dram_tensor("sinb", [64, S], BF16, kind="ExternalInput")
    trim = None
    emaskb = None
    if variant == "causal":
        trim = nc.dram_tensor("trim", [P, P], BF16, kind="ExternalInput")
    elif variant == "general":
        emaskb = nc.dram_tensor("emaskb", [S, S], BF16, kind="ExternalInput")
    part = nc.dram_tensor("part", [BS, DIM], BF16, kind="ExternalOutput")

    WCHUNK = 6 * KPC * 2 * 2 * P   # cols per ck chunk = 6144

    with tile.TileContext(nc) as tc:
        with (
            tc.tile_pool(name="const", bufs=1) as constp,
            tc.tile_pool(name="kv", bufs=1) as kvp,
            tc.tile_pool(name="qp", bufs=1) as qp,
        ):
            identb = constp.tile([P, P], BF16)
            onesb = constp.tile([P, P], BF16)
            onesrow = constp.tile([1, P], BF16)
            csp_cos = constp.tile([64, S], BF16)
            csp_sin = constp.tile([64, S], BF16)
            trim_sb = None
            if variant == "causal":
                trim_sb = constp.tile([P, P], BF16)

            # per-batch tiles: phase B(b) must not pick up false semaphore
            # dependencies on the other batch's phase-A writes
            KTb = [kvp.tile([P, S], BF16, name=f"KT{b}") for b in range(B)]
            Vtkb = [kvp.tile([P, S], BF16, name=f"Vt{b}") for b in range(B)]
            qtb = [[qp.tile([P, S], BF16, name=f"qt{h}_{b}")
                    for h in range(QH)] for b in range(B)]

            # ---------------- Phase A: QKV projection + RoPE ----------------
            with (
                tc.tile_pool(name="wqp", bufs=1) as wqp,
                tc.tile_pool(name="xtp", bufs=1) as xtp,
                tc.tile_pool(name="rp", bufs=1) as rp,
                tc.tile_pool(name="psA", bufs=1, space="PSUM") as psA,
            ):
                w_sb = wqp.tile([P, NCK * WCHUNK], F8)

                x_tiles = {}

                def _load_x(tcn, kp0, kp1):
                    if tcn not in x_tiles:
                        x_tiles[tcn] = (
                            xtp.tile([P, 2 * NKP, TCH], F8, tag="xh", bufs=2,
                                     name=f"xh_{tcn}"),
                            xtp.tile([P, 2 * NKP, TCH], F8, tag="xl", bufs=2,
                                     name=f"xl_{tcn}"),
                        )
                    xh_t, xl_t = x_tiles[tcn]
                    for src, dst in ((xh8, xh_t), (xl8, xl_t)):
                        sre = src[:, :].rearrange(
                            "(kp s p) n -> p (kp s) n", p=P, s=2)
                        nc.sync.dma_start(
                            dst[:, kp0 * 2:kp1 * 2, :],
                            sre[:, kp0 * 2:kp1 * 2,
                                tcn * TCH:(tcn + 1) * TCH])

                # startup: per-chunk (w, xh, xl) interleave so the DMA queue
                # stays just ahead of tcn0's compute; constants afterwards
                make_identity(nc, identb)
                nc.vector.memset(onesb[:], 1.0)
                nc.vector.memset(onesrow[:], 1.0)
                MW = KPC * 2 * 2 * P   # w cols per (ck, m) = 1024
                nc.sync.dma_start(w_sb[:, 0:MW], w8[:, 0:MW])
                _load_x(0, 0, 2)
                nc.sync.dma_start(w_sb[:, MW:6 * MW], w8[:, MW:6 * MW])
                for ck in range(1, NCK):
                    _load_x(0, 2 * ck, 2 * ck + 2)
                    nc.sync.dma_start(
                        w_sb[:, ck * WCHUNK:(ck + 1) * WCHUNK],
                        w8[:, ck * WCHUNK:(ck + 1) * WCHUNK])
                    if ck == 5:
                        nc.sync.dma_start(csp_cos[:], cosb[:, :])
                        nc.sync.dma_start(csp_sin[:], sinb[:, :])
                _load_x(1, 0, 4)
                if variant == "causal":
                    nc.sync.dma_start(trim_sb[:], trim[:, :])

                def _wsl(ck, m, kpi, hl):
                    off = ((((ck * 6 + m) * KPC + kpi) * 2 + hl) * 2) * P
                    return w_sb[:, off:off + 2 * P].rearrange(
                        "p (s c) -> p s c", s=2)

                for tcn in range(NCH):
                    if tcn + 1 < NCH:
                        _load_x(tcn + 1, 4 if tcn == 0 else 0, 8)
                        _load_x(tcn + 1, 8, 16)
                    xh_t, xl_t = x_tiles.pop(tcn)
                    acc = [psA.tile([P, TCH], F32, tag=f"acc{m}", bufs=1,
                                    name=f"acc{m}_{tcn}")
                           for m in range(6)]
                    morder = (5, 4, 0, 1, 2, 3) if tcn == NCH - 1 else \
                        (0, 1, 2, 3, 4, 5)
                    for ck in range(NCK):
                        last_ck = ck == NCK - 1
                        for m in morder:
                            for kpi in range(KPC):
                                kp = ck * KPC + kpi
                                rh = xh_t[:, 2 * kp:2 * kp + 2, :]
                                rl = xl_t[:, 2 * kp:2 * kp + 2, :]
                                wh = _wsl(ck, m, kpi, 0)
                                wl = _wsl(ck, m, kpi, 1)
                                st = ck == 0 and kpi == 0
                                sp = last_ck and kpi == KPC - 1
                                nc.tensor.matmul(acc[m][:], wh, rh,
                                                 start=st, stop=False,
                                                 perf_mode=DR)
                                nc.tensor.matmul(acc[m][:], wh, rl,
                                                 start=False, stop=False,
                                                 perf_mode=DR)
                                nc.tensor.matmul(acc[m][:], wl, rh,
                                                 start=False, stop=sp,
                                                 perf_mode=DR)
                            if not last_ck:
                                continue
                            # m finished all chunks: copy out + rope now,
                            # overlapping with m+1's matmuls
                            cs = csp_cos[:, (tcn % 4) * TCH:
                                         (tcn % 4 + 1) * TCH]
                            sn = csp_sin[:, (tcn % 4) * TCH:
                                         (tcn % 4 + 1) * TCH]
                            if m < 5:
                                slo = rp.tile([64, TCH], BF16, tag="slo",
                                              bufs=2, name=f"slo{m}_{tcn}")
                                shi = rp.tile([64, TCH], BF16, tag="shi",
                                              bufs=2, name=f"shi{m}_{tcn}")
                                nc.scalar.activation(slo[:], acc[m][0:64, :],
                                                     AF.Copy, scale=PSC)
                                nc.scalar.activation(shi[:], acc[m][64:P, :],
                                                     AF.Copy, scale=PSC)
                                tb, cb = tcn // 4, tcn % 4
                                if m < QH:
                                    dst = qtb[tb][m][:, cb * TCH:
                                                     (cb + 1) * TCH]
                                else:
                                    dst = KTb[tb][:, cb * TCH:(cb + 1) * TCH]
                                o_lo, o_hi = dst[0:64, :], dst[64:P, :]
                                tA = rp.tile([64, TCH], BF16, tag="tA",
                                             bufs=2, name=f"tA{m}_{tcn}")
                                tB = rp.tile([64, TCH], BF16, tag="tB",
                                             bufs=2, name=f"tB{m}_{tcn}")
                                nc.vector.tensor_mul(tA[:], slo[:], cs)
                                nc.vector.tensor_mul(tB[:], shi[:], sn)
                                nc.vector.tensor_sub(o_lo, tA[:], tB[:])
                                tC = rp.tile([64, TCH], BF16, tag="tC",
                                             bufs=2, name=f"tC{m}_{tcn}")
                                tD = rp.tile([64, TCH], BF16, tag="tD",
                                             bufs=2, name=f"tD{m}_{tcn}")
                                nc.vector.tensor_mul(tC[:], slo[:], sn)
                                nc.vector.tensor_mul(tD[:], shi[:], cs)
                                nc.vector.tensor_add(o_hi, tC[:], tD[:])
                            else:
                                vch = rp.tile([P, TCH], BF16, tag="vch",
                                              bufs=2, name=f"vch_{tcn}")
                                nc.scalar.activation(vch[:], acc[m][:],
                                                     AF.Copy, scale=PSC)
                                for j in range(TCH // P):
                                    tp_ps = psA.tile([P, P], BF16, tag="tp",
                                                     bufs=2,
                                                     name=f"tp_{tcn}_{j}")
                                    nc.tensor.transpose(
                                        tp_ps[:], vch[:, j * P:(j + 1) * P],
                                        identb[:])
                                    g = (tcn % 4) * 4 + j
                                    nc.scalar.copy(
                                        Vtkb[tcn // 4][:, g * P:(g + 1) * P],
                                        tp_ps[:])

            # ------------- Phases B+C per batch -------------
            with (
                tc.tile_pool(name="ebp", bufs=1) as ebp,
                tc.tile_pool(name="obp", bufs=1) as obp,
                tc.tile_pool(name="wop", bufs=1) as wop,
                tc.tile_pool(name="osb", bufs=1) as osbp,
                tc.tile_pool(name="mkp", bufs=1) as mkp,
            ):
                # wo pairs hi/lo: [j][hl] -> [P, 2, DIM] fp8
                wo_sb = [[None, None] for _ in range(2)]
                for j in range(2):
                    for hl in range(2):
                        t2 = wop.tile([P, 2, DIM], F8, tag=f"wo{j}{hl}",
                                      bufs=1, name=f"wo_sb{j}{hl}")
                        r0 = (j * 2 + hl) * P
                        nc.sync.dma_start(
                            t2[:],
                            wo8[r0:r0 + P, :].rearrange(
                                "p (s d) -> p s d", s=2))
                        wo_sb[j][hl] = t2

                Opairs = {}
                c_queue = []

                def emit_C_unit(b, tt, ng, pool=None, tag="pc", bufs=2,
                                tail=False, last=False):
                    # one group = 4 nn output chunks sharing one out-DMA
                    Opair = Opairs[b]
                    ob = obp.tile([P, 4 * TCH], BF16, tag="obc", bufs=6)
                    for q in range(4):
                        nn = ng * 4 + q
                        pp = (pool or psC).tile([P, TCH], F32, tag=tag,
                                                bufs=bufs)
                        for j in range(2):
                            oh = Opair[j][0][:, :, tt * P:(tt + 1) * P]
                            ol = Opair[j][1][:, :, tt * P:(tt + 1) * P]
                            wh = wo_sb[j][0][:, :, nn * TCH:(nn + 1) * TCH]
                            wl = wo_sb[j][1][:, :, nn * TCH:(nn + 1) * TCH]
                            nc.tensor.matmul(pp[:], oh, wh, start=(j == 0),
                                             stop=False, perf_mode=DR)
                            nc.tensor.matmul(pp[:], ol, wh, start=False,
                                             stop=False, perf_mode=DR)
                            nc.tensor.matmul(pp[:], oh, wl, start=False,
                                             stop=(j == 1), perf_mode=DR)
                        dst = ob[:, q * TCH:(q + 1) * TCH]
                        if tail:
                            hf = TCH // 2
                            nc.vector.tensor_scalar_mul(
                                dst[:, 0:hf], pp[:, 0:hf], PSC_C)
                            nc.scalar.activation(dst[:, hf:TCH],
                                                 pp[:, hf:TCH],
                                                 AF.Copy, scale=PSC_C)
                        elif q % 2 == 0:
                            nc.vector.tensor_scalar_mul(dst, pp[:], PSC_C)
                        else:
                            nc.scalar.activation(dst, pp[:], AF.Copy,
                                                 scale=PSC_C)
                        if last:
                            nc.sync.dma_start(
                                part[b * S + tt * P:b * S + (tt + 1) * P,
                                     nn * TCH:(nn + 1) * TCH], dst)
                    if not last:
                        nc.sync.dma_start(
                            part[b * S + tt * P:b * S + (tt + 1) * P,
                                 ng * 4 * TCH:(ng + 1) * 4 * TCH], ob[:])

                def drain_C(k):
                    for _ in range(min(k, len(c_queue))):
                        emit_C_unit(*c_queue.pop(0))

                psB_cm = tc.tile_pool(name="psB", bufs=1, space="PSUM")
                psB = psB_cm.__enter__()
                psC_cm = tc.tile_pool(name="psC", bufs=1, space="PSUM")
                psC = psC_cm.__enter__()
                for b in range(B):
                    # O pairs fp8 hi/lo for this batch: [j][hl] -> [P, 2, S]
                    Opair = [[osbp.tile([P, 2, S], F8, tag=f"op{j}{hl}_{b}",
                                        bufs=1, name=f"op{j}{hl}_{b}")
                              for hl in range(2)] for j in range(2)]
                    Opairs[b] = Opair

                    # ---- Phase B: attention for batch b, C units of the
                    # previous sc-quarter interleaved into the PE stream ----
                    for sc in range(4):
                        for h in range(QH):
                            ntt = 4 * sc + 4 if variant == "causal" else 16
                            o_ps = psB.tile([P, TCH], F32, tag="o", bufs=1,
                                            name=f"o_{b}_{h}_{sc}")
                            # sc=0 loops run uncovered at the phase boundary:
                            # a direct replicated ones-rowsum (short latency
                            # chain) beats the transposed path there
                            direct = variant == "causal" and sc == 0
                            s_direct = None
                            sT_ps = None
                            if direct:
                                s_direct = psB.tile([P, TCH], F32, tag="db",
                                                    bufs=1)
                            else:
                                # softmax denominators as [128, 1] columns
                                # (out-free=1 matmuls are ~free on PE)
                                sT_ps = psB.tile([P, 4], F32, tag="st",
                                                 bufs=1,
                                                 name=f"st_{b}_{h}_{sc}")

                            def _scores(tt):
                                diag = variant == "causal" and tt >= 4 * sc
                                c0 = (tt - 4 * sc) * P if diag else 0
                                scp = psB.tile([P, TCH], F32, tag="sc",
                                               bufs=2)
                                nc.tensor.matmul(
                                    scp[:, c0:TCH],
                                    KTb[b][:, tt * P:(tt + 1) * P],
                                    qtb[b][h][:, sc * TCH + c0:
                                              (sc + 1) * TCH],
                                    start=True, stop=True)
                                return diag, c0, scp

                            pend = _scores(0)
                            for tt in range(ntt):
                                diag, c0, sc_ps = pend
                                if tt + 1 < ntt:
                                    pend = _scores(tt + 1)
                                et = ebp.tile([P, TCH], BF16, tag="et",
                                              bufs=6)
                                nc.scalar.activation(et[:, c0:TCH],
                                                     sc_ps[:, c0:TCH], AF.Exp)
                                if diag:
                                    nc.vector.tensor_mul(
                                        et[:, c0:c0 + P], et[:, c0:c0 + P],
                                        trim_sb[:])
                                elif variant == "general":
                                    mt = mkp.tile([P, TCH], BF16, tag="mg",
                                                  bufs=3)
                                    nc.sync.dma_start(
                                        mt[:],
                                        emaskb[tt * P:(tt + 1) * P,
                                               sc * TCH:(sc + 1) * TCH])
                                    nc.vector.tensor_mul(et[:, :], et[:, :],
                                                         mt[:])
                                nc.tensor.matmul(
                                    o_ps[:, c0:TCH],
                                    Vtkb[b][:, tt * P:(tt + 1) * P],
                                    et[:, c0:TCH],
                                    start=(tt == 0), stop=(tt == ntt - 1))
                                if direct:
                                    nc.tensor.matmul(
                                        s_direct[:, c0:TCH], onesb[:],
                                        et[:, c0:TCH],
                                        start=(tt == 0),
                                        stop=(tt == ntt - 1))
                                    continue
                                jj0 = c0 // P
                                for jj in range(jj0, 4):
                                    nc.tensor.matmul(
                                        sT_ps[:, jj:jj + 1],
                                        et[:, jj * P:(jj + 1) * P],
                                        onesb[:, 0:1],
                                        start=(tt == 0 and jj == jj0),
                                        stop=(tt == ntt - 1 and jj == 3))
                            if direct:
                                db_ps = s_direct
                            else:
                                # denominators -> [1, 512] row -> broadcast
                                # to all partitions via a K=1 matmul ->
                                # reciprocal. C units drained between steps
                                # so the PE never waits on the DVE copies.
                                sT_sb = obp.tile([P, 4], BF16, tag="stsb",
                                                 bufs=2)
                                nc.vector.tensor_copy(sT_sb[:], sT_ps[:])
                                drain_C(1)
                                dT_ps = psB.tile([1, TCH], BF16, tag="dt",
                                                 bufs=1)
                                for jj in range(4):
                                    nc.tensor.matmul(
                                        dT_ps[0:1, jj * P:(jj + 1) * P],
                                        sT_sb[:, jj:jj + 1], identb[:],
                                        start=(jj == 0), stop=(jj == 3),
                                        is_transpose=True)
                                dT_sb = obp.tile([1, TCH], BF16, tag="dtsb",
                                                 bufs=2)
                                nc.vector.tensor_copy(dT_sb[:], dT_ps[:])
                                drain_C(1)
                                db_ps = psB.tile([P, TCH], F32, tag="db",
                                                 bufs=1)
                                nc.tensor.matmul(db_ps[:], onesrow[:],
                                                 dT_sb[0:1, :],
                                                 start=True, stop=True)
                            rec = obp.tile([P, TCH], F32, tag="rec", bufs=2)
                            nc.vector.reciprocal(rec[:], db_ps[:])
                            of = obp.tile([P, TCH], F32, tag="of", bufs=2)
                            nc.vector.tensor_mul(of[:], o_ps[:], rec[:])
                            j, sl = h // 2, h % 2
                            dhi = Opair[j][0][:, sl, sc * TCH:(sc + 1) * TCH]
                            dlo = Opair[j][1][:, sl, sc * TCH:(sc + 1) * TCH]
                            nc.vector.tensor_scalar_mul(dhi, of[:], SO)
                            nc.vector.scalar_tensor_tensor(
                                dlo, of[:], SO, dhi,
                                op0=ALU.mult, op1=ALU.subtract)
                            if len(c_queue) > 6:
                                drain_C(1)
                        c_queue.extend(
                            (b, tt, ng) for tt in range(4 * sc, 4 * sc + 4)
                            for ng in range(2))
                psC_cm.__exit__(None, None, None)
                psB_cm.__exit__(None, None, None)
                # final C quarter runs alone: deeper PSUM buffering to hide
                # the pp->ob copy recycle latency
                with tc.tile_pool(name="psC2", bufs=1, space="PSUM") as psC2:
                    while c_queue:
                        emit_C_unit(*c_queue.pop(0), pool=psC2, tag="pc2",
                                    bufs=8, tail=True,
                                    last=(len(c_queue) == 0))

    nc.compile()
    return nc


def _get_prog(variant):
    if variant not in _prog_cache:
        _prog_cache[variant] = _build(variant)
    return _prog_cache[variant]


_E4 = ml_dtypes.float8_e4m3


def _q8(a):
    return np.clip(a, -240.0, 240.0).astype(_E4)


def _hilo(a, scale):
    s = (a * scale).astype(np.float32)
    hi = _q8(s)
    lo = _q8(s - hi.astype(np.float32))
    return hi, lo


def prepare(inputs):
    """Host-side sharding prep: returns (variant, program, per-core maps)."""
    x = np.asarray(inputs["x"], dtype=np.float32)
    wq = np.asarray(inputs["wq"], dtype=np.float32)
    wk = np.asarray(inputs["wk"], dtype=np.float32)
    wv = np.asarray(inputs["wv"], dtype=np.float32)
    wo = np.asarray(inputs["wo"], dtype=np.float32)
    fc = np.asarray(inputs["freqs_cos"], dtype=np.float32)
    fs = np.asarray(inputs["freqs_sin"], dtype=np.float32)
    mask = np.asarray(inputs["mask"], dtype=np.float32)

    xT = x.reshape(BS, DIM).T
    xh8, xl8 = _hilo(np.ascontiguousarray(xT), SX)
    perm = np.concatenate([np.arange(0, HD, 2), np.arange(1, HD, 2)])
    wq_p = (wq.reshape(DIM, NH, HD)[:, :, perm] / math.sqrt(HD)).astype(
        np.float32)
    wk_p = wk.reshape(DIM, NKV, HD)[:, :, perm].astype(np.float32)
    cosb = np.ascontiguousarray(fc.T).astype(ml_dtypes.bfloat16)
    sinb = np.ascontiguousarray(fs.T).astype(ml_dtypes.bfloat16)

    if not mask.any():
        variant = "none"
    else:
        il, jl = np.tril_indices(S)
        iu, ju = np.triu_indices(S, 1)
        if np.all(mask[il, jl] == 0.0) and np.all(mask[iu, ju] <= -1e8):
            variant = "causal"
        else:
            variant = "general"

    trim = None
    emaskb = None
    if variant == "causal":
        # trim[t, q] = 1 if key t <= query q (within a diagonal 128-block)
        trim = np.triu(np.ones((P, P), np.float32)).astype(ml_dtypes.bfloat16)
    elif variant == "general":
        with np.errstate(under="ignore", over="ignore"):
            emaskb = np.ascontiguousarray(np.exp(mask).T).astype(
                ml_dtypes.bfloat16)

    nc = _get_prog(variant)

    in_maps = []
    for c in range(NCORES):
        # per-core weights: Q heads 4c..4c+3, KV head c
        w_eff = np.concatenate([
            wq_p[:, c * QH:(c + 1) * QH, :].reshape(DIM, DQ),
            wk_p[:, c, :],
            wv[:, c * HD:(c + 1) * HD],
        ], axis=1)                                    # [DIM, 768]
        wh, wl = _hilo(w_eff, SW)
        # pack to [P, NCK*6*KPC*2*2*P]: col ((((ck*6+m)*KPC+kpi)*2+hl)*2+s)*P+cc
        # value[hl][(2*(ck*KPC+kpi)+s)*128+p, m*128+cc]
        wpk = np.empty((P, NCK * 6 * KPC * 2 * 2 * P), dtype=_E4)
        whl = [wh.reshape(NKP, 2, P, 6, P), wl.reshape(NKP, 2, P, 6, P)]
        # index [kp, s, p, m, cc]
        for ck in range(NCK):
            for m in range(6):
                for kpi in range(KPC):
                    kp = ck * KPC + kpi
                    for hl in range(2):
                        blk = whl[hl][kp, :, :, m, :]        # [s, p, cc]
                        col0 = ((((ck * 6 + m) * KPC + kpi) * 2 + hl) * 2) * P
                        wpk[:, col0:col0 + 2 * P] = (
                            blk.transpose(1, 0, 2).reshape(P, 2 * P))
                    # wpk[p, (s,cc)] = blk[s,p,cc]
        woh, wol = _hilo(wo[c * DQ:(c + 1) * DQ, :], SW)     # [512, 4096]
        wo_pk = np.empty((4 * P, 2 * DIM), dtype=_E4)
        for j in range(2):
            for hl in range(2):
                src = (woh, wol)[hl]
                r0 = (j * 2 + hl) * P
                for s in range(2):
                    wo_pk[r0:r0 + P, s * DIM:(s + 1) * DIM] = \
                        src[(2 * j + s) * P:(2 * j + s + 1) * P, :]
        m = {
            "xh8": xh8,
            "xl8": xl8,
            "w8": wpk,
            "wo8": wo_pk,
            "cosb": cosb,
            "sinb": sinb,
        }
        if variant == "causal":
            m["trim"] = trim
        elif variant == "general":
            m["emaskb"] = emaskb
        in_maps.append(m)
    return variant, nc, in_maps


def kernel(**inputs):
    global LAST_RESULTS
    variant, nc, in_maps = prepare(inputs)
    res = run_bass_kernel_spmd(nc, in_maps, core_ids=list(range(NCORES)))
    LAST_RESULTS = res
    out = np.zeros((BS, DIM), dtype=np.float64)
    for c in range(NCORES):
        out += np.asarray(res.results[c]["part"], dtype=np.float64)
    return out.reshape(B, S, DIM).astype(np.float32)
